# revision 19
# baseline (speedup 1.0000x reference)
"""Trainium2 Bass kernel: scatter flat upper-triangular values into dense
[B, 2048, 2048] matrices (zeros below the diagonal).

Strategy (pure data parallel, 4 samples per core on 8 cores):

In the pitch-2049 "band view", out flat [2049r, 2049(r+1)) = matrix row
r's triu data (length 2048-r) followed by r+1 zeros; band-row starts are
AFFINE while input triu row offsets are quadratic (off[r] = 2048r -
r(r-1)/2). Default mode "v2trim" (stages gmbt, per-sample gathers):

  per 128-row block k (L = 2048 - 128k), per repeat:
  - 4 indirect-DMA gathers (one per sample; SWDGE, 128 descriptors of
    4L bytes each) fetch rows 128k..128k+127 into an SBUF tile [128,4,L]
  - 1 DVE tensor_tensor mult against a device-built master mask
    (m[p,x] = x < 2048-p, window 128k), broadcast over samples, zeroes
    each row's ragged tail [L-p, L)
  - 2 band stores (HWDGE, split sync/scalar by sample pair) write
    [128,2,L] to DRAM rows at stride 2049

  The below-diagonal zeros OUTSIDE the stored L-window are never
  written: both run_bass_kernel_spmd exec paths (native run_neff and
  the axon/PJRT donation redirect) pre-zero ExternalOutput buffers, a
  documented contract kernels are allowed to rely on.

Per core: ~69 MB HBM traffic (33.6 rd + 35.7 wr) ~= 193 us at the
358 GB/s per-core HBM limit; measured ~200-260 us (vs 640+ us for the
write-everything baseline kept as mode "gather").
"""

import os
import sys

import numpy as np

for _p in ("/opt/trn_rl_repo", "/opt/pypackages"):
    if _p not in sys.path and os.path.isdir(_p):
        sys.path.append(_p)

MAT = 2048
P = 128                      # partitions / rows per block
NB = MAT // P                # 16 blocks
S = 4                        # samples per core
NCORES = 8
BATCH = S * NCORES           # 32
IN_N = MAT * (MAT + 1) // 2  # 2098176 triu elements per sample
PAD = 2048
FPAD = 128                   # front pad (grouped loads read up to H before row 0)
IN_NP = FPAD + IN_N + (PAD - FPAD)  # padded per-sample input length
OUT_N = MAT * MAT
OUT_NP = OUT_N + PAD         # padded per-sample output length
ZMAX = P * (NB - 1) + 1      # max zero-parallelogram row length (1921)
G = 16                       # rows per affine load group (grouped mode)
NG = P // G                  # 8 groups per block
H = (G - 1) * (G - 2) // 2   # 105: max residual head misalignment
WM = MAT + P * (NB - 1) + H + 7   # master mask width (4080)
WT = MAT + 1 + H             # band tile width in grouped mode (2154)

_row_off = None


def _offsets():
    global _row_off
    if _row_off is None:
        r = np.arange(MAT, dtype=np.int64)
        _row_off = r * MAT - r * (r - 1) // 2
    return _row_off


def _build_nc(repeat: int = 1, stages: str = "gmsz", fold: bool = False,
              bufs: int = 3, mode: str = "gather", leng: str = "pool",
              timing: bool = False):
    """stages: g=gathers/loads, m=mask, s=band stores, z=zero fills.
    mode: "gather" (indirect-DMA gather), "grouped" (affine group loads),
    "v2full"/"v2trim" (fold-gather + DVE select + full/trim pitch stores).
    timing=True makes `out` an Internal DRAM scratch with a tiny external
    output, so repeated timing calls don't allocate/return 536 MB."""
    import concourse.bass as bass
    import concourse.tile as tile
    from concourse import bacc, mybir

    off = _offsets()
    nc = bacc.Bacc("TRN2", target_bir_lowering=False, debug=False)
    inp = nc.dram_tensor("inp", [S * IN_NP, 1], mybir.dt.float32, kind="ExternalInput")
    idxt = nc.dram_tensor("idx", [P, NB * S], mybir.dt.int32, kind="ExternalInput")
    mskt = nc.dram_tensor("msk", [P, WM], mybir.dt.float32, kind="ExternalInput")
    if timing:
        out = nc.dram_tensor("out", [S * OUT_NP], mybir.dt.float32,
                             kind="Internal")
        tiny = nc.dram_tensor("tiny", [1, 16], mybir.dt.float32,
                              kind="ExternalOutput")
    else:
        out = nc.dram_tensor("out", [S * OUT_NP], mybir.dt.float32,
                             kind="ExternalOutput")
        tiny = None

    if mode.startswith("v2"):
        return _build_v2(nc, bass, tile, mybir, inp, idxt, out, tiny,
                         repeat, stages, bufs, leng,
                         full=(mode == "v2full"))

    if mode == "grouped":
        return _build_grouped(nc, bass, tile, mybir, inp, mskt, out, off,
                              repeat, stages, bufs, leng, tiny)

    with tile.TileContext(nc) as tc:
        with (
            tc.tile_pool(name="band", bufs=bufs) as pool,
            tc.tile_pool(name="const", bufs=1) as cpool,
        ):
            idx_tile = cpool.tile([P, NB * S], mybir.dt.int32)
            nc.sync.dma_start(idx_tile[:], idxt[:, :])
            if "z" in stages:
                zt = cpool.tile([P, S * ZMAX], mybir.dt.float32)
                nc.vector.memset(zt[:], 0.0)
            for k in [k for _ in range(repeat) for k in range(NB)]:
                L = MAT - P * k
                t = pool.tile([P, S, L], mybir.dt.float32, tag="band")
                Lg = L // 4 if "q" in stages else L
                if "g" in stages:
                    if fold:
                        nc.gpsimd.indirect_dma_start(
                            out=t[:],
                            out_offset=None,
                            in_=inp[:],
                            in_offset=bass.IndirectOffsetOnAxis(
                                ap=idx_tile[:, k * S:(k + 1) * S], axis=0
                            ),
                        )
                    else:
                        for s in range(S):
                            nc.gpsimd.indirect_dma_start(
                                out=t[:, s, :Lg],
                                out_offset=None,
                                in_=inp[:],
                                in_offset=bass.IndirectOffsetOnAxis(
                                    ap=idx_tile[:, k * S + s:k * S + s + 1], axis=0
                                ),
                            )
                if "c" in stages:
                    # control: plain contiguous load of the same byte count
                    cap = bass.AP(inp, 0, [[S * L, P], [1, S * L]])
                    nc.sync.dma_start(out=t[:], in_=cap)
                if "m" in stages:
                    # keep element (p, s, l) iff l < L - p (the row's data len)
                    nc.gpsimd.affine_select(
                        out=t[:],
                        in_=t[:],
                        compare_op=mybir.AluOpType.is_gt,
                        fill=0.0,
                        base=L,
                        pattern=[[0, S], [-1, L]],
                        channel_multiplier=-1,
                    )
                if "s" in stages:
                    # band store: band row p -> flat 2049*(128k+p), per sample
                    oap = bass.AP(
                        out, (MAT + 1) * P * k, [[MAT + 1, P], [OUT_NP, S], [1, L]]
                    )
                    nc.sync.dma_start(out=oap, in_=t[:])
                if "z" in stages:
                    # zero parallelogram: matrix rows R=128k+1+j (j<cnt),
                    # cols [R-1-128k, R-1], length 128k+1, row starts affine
                    zl = P * k + 1
                    cnt = P if k < NB - 1 else P - 1
                    zap = bass.AP(
                        out,
                        (P * k + 1) * MAT,
                        [[MAT + 1, cnt], [OUT_NP, S], [1, zl]],
                    )
                    nc.scalar.dma_start(out=zap, in_=zt[:cnt, :S * zl])
            if tiny is not None:
                tt = cpool.tile([1, 16], mybir.dt.float32)
                nc.vector.memset(tt[:], 0.0)
                nc.sync.dma_start(out=tiny[:, :], in_=tt[:])
    nc.compile()
    return nc


def _build_v2(nc, bass, tile, mybir, inp, idxt, out, tiny,
              repeat, stages, bufs, leng, full):
    """Indirect gathers + DVE mask multiply + pitch-2049 band stores.

    full=True: store the whole 2049-wide band row (data + zeros) -> covers
    every output byte, one contiguous 1 MB DRAM range per (block, sample).
    full=False ("trim"): store only [0, L) per band row; the below-diagonal
    zeros outside that window come from the pre-zeroed (donated) output
    buffer that run_bass_kernel_spmd supplies.
    leng: "four" = 4 per-sample indirect gathers per block (use this;
          "fold" = 1 gather with a [P, S] offset AP, which CRASHES the
          device - NRT_EXEC_UNIT_UNRECOVERABLE - despite passing CoreSim).
    stages: b = broadcast mask (1 DVE mult/block), t = stores split
    across sync+scalar by sample pair, T = per-sample stores.
    """
    WV = P * (NB - 1) + MAT + 1  # 3969: master-mask width for v2
    with tile.TileContext(nc) as tc:
        with (
            tc.tile_pool(name="band", bufs=bufs) as pool,
            tc.tile_pool(name="const", bufs=1) as cpool,
        ):
            idx_tile = cpool.tile([P, NB * S], mybir.dt.int32)
            nc.sync.dma_start(idx_tile[:], idxt[:, :])
            if "m" in stages:
                # master mask built on-device: m[p, x] = 1 iff x < MAT - p
                mk = cpool.tile([P, WV], mybir.dt.float32)
                nc.vector.memset(mk[:], 1.0)
                nc.gpsimd.affine_select(
                    out=mk[:], in_=mk[:],
                    compare_op=mybir.AluOpType.is_gt,
                    fill=0.0, base=MAT,
                    pattern=[[-1, WV]], channel_multiplier=-1,
                )
            for k in [k for _ in range(repeat) for k in range(NB)]:
                L = MAT - P * k
                W = MAT + 1 if full else L
                t = pool.tile([P, S, W], mybir.dt.float32, tag="band")
                if "g" in stages:
                    if leng == "four":
                        for s in range(S):
                            nc.gpsimd.indirect_dma_start(
                                out=t[:, s, :L],
                                out_offset=None,
                                in_=inp[:],
                                in_offset=bass.IndirectOffsetOnAxis(
                                    ap=idx_tile[:, k * S + s:k * S + s + 1],
                                    axis=0,
                                ),
                            )
                    else:
                        nc.gpsimd.indirect_dma_start(
                            out=t[:, :, :L],
                            out_offset=None,
                            in_=inp[:],
                            in_offset=bass.IndirectOffsetOnAxis(
                                ap=idx_tile[:, k * S:(k + 1) * S], axis=0
                            ),
                        )
                if full and W > L and "s" in stages:
                    # tail zeros [L, 2049): every stored byte is written by
                    # THIS tile (no stale-slot reads -> race-free under Tile)
                    nc.gpsimd.memset(t[:, :, L:W], 0.0)
                if "M" in stages:
                    # gpsimd variant: keep (p, s, l) iff l < L - p
                    nc.gpsimd.affine_select(
                        out=t[:, :, :L],
                        in_=t[:, :, :L],
                        compare_op=mybir.AluOpType.is_gt,
                        fill=0.0,
                        base=L,
                        pattern=[[0, S], [-1, L]],
                        channel_multiplier=-1,
                    )
                elif "b" in stages and "m" in stages:
                    # one DVE mult per block: mask broadcast over samples
                    nc.vector.tensor_tensor(
                        out=t[:, :, :L],
                        in0=t[:, :, :L],
                        in1=mk[:, P * k:P * k + L].unsqueeze(1)
                            .to_broadcast([P, S, L]),
                        op=mybir.AluOpType.mult,
                    )
                elif "m" in stages:
                    # DVE: t[p, s, l] *= m[p, 128k + l]  (1 iff l < L - p)
                    for s in range(S):
                        nc.vector.tensor_tensor(
                            out=t[:, s, :L],
                            in0=t[:, s, :L],
                            in1=mk[:, P * k:P * k + L],
                            op=mybir.AluOpType.mult,
                        )
                if "T" in stages:
                    # per-sample stores, engines alternating
                    for s in range(S):
                        oap = bass.AP(
                            out, (MAT + 1) * P * k + OUT_NP * s,
                            [[MAT + 1, P], [1, W]],
                        )
                        eng = nc.sync if s % 2 == 0 else nc.scalar
                        eng.dma_start(out=oap, in_=t[:, s, :])
                elif "t" in stages:
                    # split store across both HWDGE engines by sample pairs
                    for h in range(2):
                        oap = bass.AP(
                            out, (MAT + 1) * P * k + (OUT_NP * 2) * h,
                            [[MAT + 1, P], [OUT_NP, 2], [1, W]],
                        )
                        eng = nc.sync if h == 0 else nc.scalar
                        eng.dma_start(out=oap, in_=t[:, 2 * h:2 * h + 2, :])
                elif "s" in stages:
                    oap = bass.AP(
                        out, (MAT + 1) * P * k,
                        [[MAT + 1, P], [OUT_NP, S], [1, W]],
                    )
                    eng = nc.sync if k % 2 == 0 else nc.scalar
                    eng.dma_start(out=oap, in_=t[:])
            if tiny is not None:
                tt = cpool.tile([1, 16], mybir.dt.float32)
                nc.vector.memset(tt[:], 0.0)
                nc.sync.dma_start(out=tiny[:, :], in_=tt[:])
    nc.compile()
    return nc


def _build_grouped(nc, bass, tile, mybir, inp, mskt, out, off,
                   repeat, stages, bufs, leng="pool", tiny=None):
    """Affine-only pipeline: per block, NG affine group loads (16 rows at
    constant stride L-16a, head-misaligned by h(b)=H-b(b-1)/2), one mask
    multiply per sample against a sliding master mask, then per-b-class
    band stores whose SBUF column offset h(b) absorbs the misalignment."""
    with tile.TileContext(nc) as tc:
        with (
            tc.tile_pool(name="band", bufs=bufs) as pool,
            tc.tile_pool(name="const", bufs=1) as cpool,
        ):
            msk_tile = cpool.tile([P, WM], mybir.dt.float32)
            nc.sync.dma_start(msk_tile[:], mskt[:, :])
            if "z" in stages:
                zt = cpool.tile([P, S * ZMAX], mybir.dt.float32)
                nc.vector.memset(zt[:], 0.0)
            # pre-zero the band slots so stale-bit NaNs can't leak through
            # the mask multiply (0 * NaN = NaN)
            ext = "x" in stages
            tw = WT if ext else MAT + H
            for _ in range(bufs):
                t0 = pool.tile([P, S, tw], mybir.dt.float32, tag="band")
                nc.vector.memset(t0[:], 0.0)
            for k in [k for _ in range(repeat) for k in range(NB)]:
                L = MAT - P * k
                W = WT if ext else L + H
                r0 = P * k
                t = pool.tile([P, S, W], mybir.dt.float32, tag="band")
                if "g" in stages:
                    for a in range(NG):
                        Lc = L + H - G * a
                        start = FPAD + int(off[r0 + G * a]) - H
                        iap = bass.AP(
                            inp, start,
                            [[L - G * a, G], [IN_NP, S], [1, Lc]],
                        )
                        le = (nc.gpsimd if leng == "pool"
                              else (nc.sync if a % 2 == 0 else nc.scalar))
                        le.dma_start(out=t[G * a:G * (a + 1), :, :Lc],
                                     in_=iap)
                if "m" in stages:
                    for s in range(S):
                        nc.vector.tensor_tensor(
                            out=t[:, s, :],
                            in0=t[:, s, :],
                            in1=msk_tile[:, P * k:P * k + W],
                            op=mybir.AluOpType.mult,
                        )
                if ext:
                    # full-pitch stores: band row r covers out flat
                    # [2049r, 2049(r+1)) = row r data + row r+1 zero prefix
                    # (incl. subdiagonal); union over r tiles the padded
                    # output exactly -> no separate zero fills
                    for b in range(G):
                        h = H - b * (b - 1) // 2
                        sb = t[b::G, :, h:h + MAT + 1]
                        oap = bass.AP(
                            out, (MAT + 1) * (r0 + b),
                            [[(MAT + 1) * G, NG], [OUT_NP, S], [1, MAT + 1]],
                        )
                        eng = nc.sync if b % 2 == 0 else nc.scalar
                        eng.dma_start(out=oap, in_=sb)
                elif "s" in stages:
                    for b in range(G):
                        h = H - b * (b - 1) // 2
                        sb = t[b::G, :, h:h + L]
                        oap = bass.AP(
                            out, (MAT + 1) * (r0 + b),
                            [[(MAT + 1) * G, NG], [OUT_NP, S], [1, L]],
                        )
                        eng = nc.sync if b % 2 == 0 else nc.scalar
                        eng.dma_start(out=oap, in_=sb)
                if "z" in stages:
                    zl = P * k + 1
                    cnt = P if k < NB - 1 else P - 1
                    zap = bass.AP(
                        out, (P * k + 1) * MAT,
                        [[MAT + 1, cnt], [OUT_NP, S], [1, zl]],
                    )
                    nc.scalar.dma_start(out=zap, in_=zt[:cnt, :S * zl])
            if tiny is not None:
                tt = cpool.tile([1, 16], mybir.dt.float32)
                nc.vector.memset(tt[:], 0.0)
                nc.sync.dma_start(out=tiny[:, :], in_=tt[:])
    nc.compile()
    return nc


MODE = os.environ.get("TRIU_MODE", "v2trim")

_NC = None


def _default_build(repeat: int = 1, timing: bool = False):
    if MODE.startswith("v2"):
        stages, leng, bufs = "gmbt", "four", 4
    elif MODE == "grouped":
        stages, leng, bufs = "gmx", "pool", 3
    else:
        stages, leng, bufs = "gmsz", "pool", 3
    return _build_nc(repeat=repeat, mode=MODE, stages=stages, leng=leng,
                     bufs=bufs, timing=timing)


def _get_nc():
    global _NC
    if _NC is None:
        _NC = _default_build()
    return _NC


def _mask_array() -> np.ndarray:
    # master mask: m[p, x] = 1 iff x < MAT + H - p - D16(p % G)
    p = np.arange(P)[:, None]
    x = np.arange(WM)[None, :]
    b = p % G
    thr = MAT + H - p - b * (b - 1) // 2
    return (x < thr).astype(np.float32)


def make_in_maps(inputs: np.ndarray):
    """Shard + pad the [32, IN_N] input into 8 per-core in_maps."""
    assert inputs.shape == (BATCH, IN_N), inputs.shape
    x = np.ascontiguousarray(inputs, dtype=np.float32)
    xp = np.zeros((BATCH, IN_NP), dtype=np.float32)
    xp[:, FPAD:FPAD + IN_N] = x
    xp = xp.reshape(NCORES, S * IN_NP)

    off = _offsets()
    idx = np.zeros((P, NB * S), dtype=np.int32)
    for k in range(NB):
        for s in range(S):
            idx[:, k * S + s] = (
                FPAD + off[k * P:(k + 1) * P] + s * IN_NP
            ).astype(np.int32)
    msk = _mask_array()
    return [{"inp": xp[c][:, None], "idx": idx, "msk": msk}
            for c in range(NCORES)]


def assemble_out(results) -> np.ndarray:
    outs = []
    for c in range(NCORES):
        o = results[c]["out"].reshape(S, OUT_NP)[:, :OUT_N]
        outs.append(o.reshape(S, MAT, MAT))
    return np.concatenate(outs, axis=0)


def kernel(inputs: np.ndarray) -> np.ndarray:
    from concourse.bass_utils import run_bass_kernel_spmd

    nc = _get_nc()
    in_maps = make_in_maps(np.asarray(inputs))
    res = run_bass_kernel_spmd(nc, in_maps, core_ids=list(range(NCORES)))
    return assemble_out(res.results)


if __name__ == "__main__":
    rng = np.random.default_rng(0)
    x = rng.standard_normal((BATCH, IN_N), dtype=np.float32)
    y = kernel(x)
    # numpy reference
    r, c = np.triu_indices(MAT)
    exp = np.zeros((BATCH, MAT, MAT), dtype=np.float32)
    exp[:, r, c] = x
    err = np.abs(y - exp).max()
    denom = max(np.abs(exp).max(), 1e-9)
    print("max abs err:", err, "rel:", err / denom)
    assert err == 0.0, "mismatch"
    print("OK")



# revision 20
# speedup vs baseline: 1.0909x; 1.0909x over previous
"""Trainium2 Bass kernel: scatter flat upper-triangular values into dense
[B, 2048, 2048] matrices (zeros below the diagonal).

Strategy (pure data parallel, 4 samples per core on 8 cores):

In the pitch-2049 "band view", out flat [2049r, 2049(r+1)) = matrix row
r's triu data (length 2048-r) followed by r+1 zeros; band-row starts are
AFFINE while input triu row offsets are quadratic (off[r] = 2048r -
r(r-1)/2). Default mode "v2trim" (stages gmbt, per-sample gathers):

  per 128-row block k (L = 2048 - 128k), per repeat:
  - 4 indirect-DMA gathers (one per sample; SWDGE, 128 descriptors of
    4L bytes each) fetch rows 128k..128k+127 into an SBUF tile [128,4,L]
  - 1 DVE tensor_tensor mult against a device-built master mask
    (m[p,x] = x < 2048-p, window 128k), broadcast over samples, zeroes
    each row's ragged tail [L-p, L)
  - 2 band stores (HWDGE, split sync/scalar by sample pair) write
    [128,2,L] to DRAM rows at stride 2049

  The below-diagonal zeros OUTSIDE the stored L-window are never
  written: both run_bass_kernel_spmd exec paths (native run_neff and
  the axon/PJRT donation redirect) pre-zero ExternalOutput buffers, a
  documented contract kernels are allowed to rely on.

Per core: ~69 MB HBM traffic (33.6 rd + 35.7 wr) ~= 193 us at the
358 GB/s per-core HBM limit; measured ~200-260 us (vs 640+ us for the
write-everything baseline kept as mode "gather").
"""

import os
import sys

import numpy as np

for _p in ("/opt/trn_rl_repo", "/opt/pypackages"):
    if _p not in sys.path and os.path.isdir(_p):
        sys.path.append(_p)

MAT = 2048
P = 128                      # partitions / rows per block
NB = MAT // P                # 16 blocks
S = 4                        # samples per core
NCORES = 8
BATCH = S * NCORES           # 32
IN_N = MAT * (MAT + 1) // 2  # 2098176 triu elements per sample
PAD = 2048
FPAD = 128                   # front pad (grouped loads read up to H before row 0)
IN_NP = FPAD + IN_N + (PAD - FPAD)  # padded per-sample input length
OUT_N = MAT * MAT
OUT_NP = OUT_N + PAD         # padded per-sample output length
ZMAX = P * (NB - 1) + 1      # max zero-parallelogram row length (1921)
G = 16                       # rows per affine load group (grouped mode)
NG = P // G                  # 8 groups per block
H = (G - 1) * (G - 2) // 2   # 105: max residual head misalignment
WM = MAT + P * (NB - 1) + H + 7   # master mask width (4080)
WT = MAT + 1 + H             # band tile width in grouped mode (2154)

_row_off = None


def _offsets():
    global _row_off
    if _row_off is None:
        r = np.arange(MAT, dtype=np.int64)
        _row_off = r * MAT - r * (r - 1) // 2
    return _row_off


def _build_nc(repeat: int = 1, stages: str = "gmsz", fold: bool = False,
              bufs: int = 3, mode: str = "gather", leng: str = "pool",
              timing: bool = False):
    """stages: g=gathers/loads, m=mask, s=band stores, z=zero fills.
    mode: "gather" (indirect-DMA gather), "grouped" (affine group loads),
    "v2full"/"v2trim" (fold-gather + DVE select + full/trim pitch stores).
    timing=True makes `out` an Internal DRAM scratch with a tiny external
    output, so repeated timing calls don't allocate/return 536 MB."""
    import concourse.bass as bass
    import concourse.tile as tile
    from concourse import bacc, mybir

    off = _offsets()
    nc = bacc.Bacc("TRN2", target_bir_lowering=False, debug=False)
    inp = nc.dram_tensor("inp", [S * IN_NP, 1], mybir.dt.float32, kind="ExternalInput")
    idxt = nc.dram_tensor("idx", [P, NB * S], mybir.dt.int32, kind="ExternalInput")
    mskt = nc.dram_tensor("msk", [P, WM], mybir.dt.float32, kind="ExternalInput")
    if timing:
        out = nc.dram_tensor("out", [S * OUT_NP], mybir.dt.float32,
                             kind="Internal")
        tiny = nc.dram_tensor("tiny", [1, 16], mybir.dt.float32,
                              kind="ExternalOutput")
    else:
        out = nc.dram_tensor("out", [S * OUT_NP], mybir.dt.float32,
                             kind="ExternalOutput")
        tiny = None

    if mode.startswith("v2"):
        return _build_v2(nc, bass, tile, mybir, inp, idxt, out, tiny,
                         repeat, stages, bufs, leng,
                         full=(mode == "v2full"))

    if mode == "grouped":
        return _build_grouped(nc, bass, tile, mybir, inp, mskt, out, off,
                              repeat, stages, bufs, leng, tiny)

    with tile.TileContext(nc) as tc:
        with (
            tc.tile_pool(name="band", bufs=bufs) as pool,
            tc.tile_pool(name="const", bufs=1) as cpool,
        ):
            idx_tile = cpool.tile([P, NB * S], mybir.dt.int32)
            nc.sync.dma_start(idx_tile[:], idxt[:, :])
            if "z" in stages:
                zt = cpool.tile([P, S * ZMAX], mybir.dt.float32)
                nc.vector.memset(zt[:], 0.0)
            for k in [k for _ in range(repeat) for k in range(NB)]:
                L = MAT - P * k
                t = pool.tile([P, S, L], mybir.dt.float32, tag="band")
                Lg = L // 4 if "q" in stages else L
                if "g" in stages:
                    if fold:
                        nc.gpsimd.indirect_dma_start(
                            out=t[:],
                            out_offset=None,
                            in_=inp[:],
                            in_offset=bass.IndirectOffsetOnAxis(
                                ap=idx_tile[:, k * S:(k + 1) * S], axis=0
                            ),
                        )
                    else:
                        for s in range(S):
                            nc.gpsimd.indirect_dma_start(
                                out=t[:, s, :Lg],
                                out_offset=None,
                                in_=inp[:],
                                in_offset=bass.IndirectOffsetOnAxis(
                                    ap=idx_tile[:, k * S + s:k * S + s + 1], axis=0
                                ),
                            )
                if "c" in stages:
                    # control: plain contiguous load of the same byte count
                    cap = bass.AP(inp, 0, [[S * L, P], [1, S * L]])
                    nc.sync.dma_start(out=t[:], in_=cap)
                if "m" in stages:
                    # keep element (p, s, l) iff l < L - p (the row's data len)
                    nc.gpsimd.affine_select(
                        out=t[:],
                        in_=t[:],
                        compare_op=mybir.AluOpType.is_gt,
                        fill=0.0,
                        base=L,
                        pattern=[[0, S], [-1, L]],
                        channel_multiplier=-1,
                    )
                if "s" in stages:
                    # band store: band row p -> flat 2049*(128k+p), per sample
                    oap = bass.AP(
                        out, (MAT + 1) * P * k, [[MAT + 1, P], [OUT_NP, S], [1, L]]
                    )
                    nc.sync.dma_start(out=oap, in_=t[:])
                if "z" in stages:
                    # zero parallelogram: matrix rows R=128k+1+j (j<cnt),
                    # cols [R-1-128k, R-1], length 128k+1, row starts affine
                    zl = P * k + 1
                    cnt = P if k < NB - 1 else P - 1
                    zap = bass.AP(
                        out,
                        (P * k + 1) * MAT,
                        [[MAT + 1, cnt], [OUT_NP, S], [1, zl]],
                    )
                    nc.scalar.dma_start(out=zap, in_=zt[:cnt, :S * zl])
            if tiny is not None:
                tt = cpool.tile([1, 16], mybir.dt.float32)
                nc.vector.memset(tt[:], 0.0)
                nc.sync.dma_start(out=tiny[:, :], in_=tt[:])
    nc.compile()
    return nc


def _build_v2(nc, bass, tile, mybir, inp, idxt, out, tiny,
              repeat, stages, bufs, leng, full):
    """Indirect gathers + DVE mask multiply + pitch-2049 band stores.

    full=True: store the whole 2049-wide band row (data + zeros) -> covers
    every output byte, one contiguous 1 MB DRAM range per (block, sample).
    full=False ("trim"): store only [0, L) per band row; the below-diagonal
    zeros outside that window come from the pre-zeroed (donated) output
    buffer that run_bass_kernel_spmd supplies.
    leng: "four" = 4 per-sample indirect gathers per block (use this;
          "fold" = 1 gather with a [P, S] offset AP, which CRASHES the
          device - NRT_EXEC_UNIT_UNRECOVERABLE - despite passing CoreSim).
    stages: b = broadcast mask (1 DVE mult/block), t = stores split
    across sync+scalar by sample pair, T = per-sample stores.
    """
    WV = P * (NB - 1) + MAT + 1  # 3969: master-mask width for v2
    with tile.TileContext(nc) as tc:
        with (
            tc.tile_pool(name="band", bufs=bufs) as pool,
            tc.tile_pool(name="const", bufs=1) as cpool,
        ):
            idx_tile = cpool.tile([P, NB * S], mybir.dt.int32)
            nc.sync.dma_start(idx_tile[:], idxt[:, :])
            if "m" in stages:
                # master mask built on-device: m[p, x] = 1 iff x < MAT - p
                mk = cpool.tile([P, WV], mybir.dt.float32)
                nc.vector.memset(mk[:], 1.0)
                nc.gpsimd.affine_select(
                    out=mk[:], in_=mk[:],
                    compare_op=mybir.AluOpType.is_gt,
                    fill=0.0, base=MAT,
                    pattern=[[-1, WV]], channel_multiplier=-1,
                )
            for k in [k for _ in range(repeat) for k in range(NB)]:
                L = MAT - P * k
                W = MAT + 1 if full else L
                t = pool.tile([P, S, W], mybir.dt.float32, tag="band")
                if "g" in stages:
                    if leng == "four":
                        for s in range(S):
                            nc.gpsimd.indirect_dma_start(
                                out=t[:, s, :L],
                                out_offset=None,
                                in_=inp[:],
                                in_offset=bass.IndirectOffsetOnAxis(
                                    ap=idx_tile[:, k * S + s:k * S + s + 1],
                                    axis=0,
                                ),
                            )
                    else:
                        nc.gpsimd.indirect_dma_start(
                            out=t[:, :, :L],
                            out_offset=None,
                            in_=inp[:],
                            in_offset=bass.IndirectOffsetOnAxis(
                                ap=idx_tile[:, k * S:(k + 1) * S], axis=0
                            ),
                        )
                if full and W > L and "s" in stages:
                    # tail zeros [L, 2049): every stored byte is written by
                    # THIS tile (no stale-slot reads -> race-free under Tile)
                    nc.gpsimd.memset(t[:, :, L:W], 0.0)
                if "M" in stages:
                    # gpsimd variant: keep (p, s, l) iff l < L - p
                    nc.gpsimd.affine_select(
                        out=t[:, :, :L],
                        in_=t[:, :, :L],
                        compare_op=mybir.AluOpType.is_gt,
                        fill=0.0,
                        base=L,
                        pattern=[[0, S], [-1, L]],
                        channel_multiplier=-1,
                    )
                elif "b" in stages and "m" in stages:
                    # one DVE mult per block: mask broadcast over samples
                    nc.vector.tensor_tensor(
                        out=t[:, :, :L],
                        in0=t[:, :, :L],
                        in1=mk[:, P * k:P * k + L].unsqueeze(1)
                            .to_broadcast([P, S, L]),
                        op=mybir.AluOpType.mult,
                    )
                elif "m" in stages:
                    # DVE: t[p, s, l] *= m[p, 128k + l]  (1 iff l < L - p)
                    for s in range(S):
                        nc.vector.tensor_tensor(
                            out=t[:, s, :L],
                            in0=t[:, s, :L],
                            in1=mk[:, P * k:P * k + L],
                            op=mybir.AluOpType.mult,
                        )
                if "T" in stages:
                    # per-sample stores, engines alternating
                    for s in range(S):
                        oap = bass.AP(
                            out, (MAT + 1) * P * k + OUT_NP * s,
                            [[MAT + 1, P], [1, W]],
                        )
                        eng = nc.sync if s % 2 == 0 else nc.scalar
                        eng.dma_start(out=oap, in_=t[:, s, :])
                elif "t" in stages:
                    # split store across both HWDGE engines by sample pairs
                    for h in range(2):
                        oap = bass.AP(
                            out, (MAT + 1) * P * k + (OUT_NP * 2) * h,
                            [[MAT + 1, P], [OUT_NP, 2], [1, W]],
                        )
                        eng = nc.sync if h == 0 else nc.scalar
                        eng.dma_start(out=oap, in_=t[:, 2 * h:2 * h + 2, :])
                elif "s" in stages:
                    oap = bass.AP(
                        out, (MAT + 1) * P * k,
                        [[MAT + 1, P], [OUT_NP, S], [1, W]],
                    )
                    eng = nc.sync if k % 2 == 0 else nc.scalar
                    eng.dma_start(out=oap, in_=t[:])
            if tiny is not None:
                tt = cpool.tile([1, 16], mybir.dt.float32)
                nc.vector.memset(tt[:], 0.0)
                nc.sync.dma_start(out=tiny[:, :], in_=tt[:])
    nc.compile()
    return nc


def _build_grouped(nc, bass, tile, mybir, inp, mskt, out, off,
                   repeat, stages, bufs, leng="pool", tiny=None):
    """Affine-only pipeline: per block, NG affine group loads (16 rows at
    constant stride L-16a, head-misaligned by h(b)=H-b(b-1)/2), one mask
    multiply per sample against a sliding master mask, then per-b-class
    band stores whose SBUF column offset h(b) absorbs the misalignment."""
    with tile.TileContext(nc) as tc:
        with (
            tc.tile_pool(name="band", bufs=bufs) as pool,
            tc.tile_pool(name="const", bufs=1) as cpool,
        ):
            msk_tile = cpool.tile([P, WM], mybir.dt.float32)
            nc.sync.dma_start(msk_tile[:], mskt[:, :])
            if "z" in stages:
                zt = cpool.tile([P, S * ZMAX], mybir.dt.float32)
                nc.vector.memset(zt[:], 0.0)
            # pre-zero the band slots so stale-bit NaNs can't leak through
            # the mask multiply (0 * NaN = NaN)
            ext = "x" in stages
            tw = WT if ext else MAT + H
            for _ in range(bufs):
                t0 = pool.tile([P, S, tw], mybir.dt.float32, tag="band")
                nc.vector.memset(t0[:], 0.0)
            for k in [k for _ in range(repeat) for k in range(NB)]:
                L = MAT - P * k
                W = WT if ext else L + H
                r0 = P * k
                t = pool.tile([P, S, W], mybir.dt.float32, tag="band")
                if "g" in stages:
                    for a in range(NG):
                        Lc = L + H - G * a
                        start = FPAD + int(off[r0 + G * a]) - H
                        iap = bass.AP(
                            inp, start,
                            [[L - G * a, G], [IN_NP, S], [1, Lc]],
                        )
                        le = (nc.gpsimd if leng == "pool"
                              else (nc.sync if a % 2 == 0 else nc.scalar))
                        le.dma_start(out=t[G * a:G * (a + 1), :, :Lc],
                                     in_=iap)
                if "m" in stages:
                    for s in range(S):
                        nc.vector.tensor_tensor(
                            out=t[:, s, :],
                            in0=t[:, s, :],
                            in1=msk_tile[:, P * k:P * k + W],
                            op=mybir.AluOpType.mult,
                        )
                if ext:
                    # full-pitch stores: band row r covers out flat
                    # [2049r, 2049(r+1)) = row r data + row r+1 zero prefix
                    # (incl. subdiagonal); union over r tiles the padded
                    # output exactly -> no separate zero fills
                    for b in range(G):
                        h = H - b * (b - 1) // 2
                        sb = t[b::G, :, h:h + MAT + 1]
                        oap = bass.AP(
                            out, (MAT + 1) * (r0 + b),
                            [[(MAT + 1) * G, NG], [OUT_NP, S], [1, MAT + 1]],
                        )
                        eng = nc.sync if b % 2 == 0 else nc.scalar
                        eng.dma_start(out=oap, in_=sb)
                elif "s" in stages:
                    for b in range(G):
                        h = H - b * (b - 1) // 2
                        sb = t[b::G, :, h:h + L]
                        oap = bass.AP(
                            out, (MAT + 1) * (r0 + b),
                            [[(MAT + 1) * G, NG], [OUT_NP, S], [1, L]],
                        )
                        eng = nc.sync if b % 2 == 0 else nc.scalar
                        eng.dma_start(out=oap, in_=sb)
                if "z" in stages:
                    zl = P * k + 1
                    cnt = P if k < NB - 1 else P - 1
                    zap = bass.AP(
                        out, (P * k + 1) * MAT,
                        [[MAT + 1, cnt], [OUT_NP, S], [1, zl]],
                    )
                    nc.scalar.dma_start(out=zap, in_=zt[:cnt, :S * zl])
            if tiny is not None:
                tt = cpool.tile([1, 16], mybir.dt.float32)
                nc.vector.memset(tt[:], 0.0)
                nc.sync.dma_start(out=tiny[:, :], in_=tt[:])
    nc.compile()
    return nc


MODE = os.environ.get("TRIU_MODE", "v2trim")

_NC = None


def _default_build(repeat: int = 1, timing: bool = False):
    if MODE.startswith("v2"):
        stages, leng, bufs = "gmbt", "four", 3
    elif MODE == "grouped":
        stages, leng, bufs = "gmx", "pool", 3
    else:
        stages, leng, bufs = "gmsz", "pool", 3
    return _build_nc(repeat=repeat, mode=MODE, stages=stages, leng=leng,
                     bufs=bufs, timing=timing)


def _get_nc():
    global _NC
    if _NC is None:
        _NC = _default_build()
    return _NC


def _mask_array() -> np.ndarray:
    # master mask: m[p, x] = 1 iff x < MAT + H - p - D16(p % G)
    p = np.arange(P)[:, None]
    x = np.arange(WM)[None, :]
    b = p % G
    thr = MAT + H - p - b * (b - 1) // 2
    return (x < thr).astype(np.float32)


def make_in_maps(inputs: np.ndarray):
    """Shard + pad the [32, IN_N] input into 8 per-core in_maps."""
    assert inputs.shape == (BATCH, IN_N), inputs.shape
    x = np.ascontiguousarray(inputs, dtype=np.float32)
    xp = np.zeros((BATCH, IN_NP), dtype=np.float32)
    xp[:, FPAD:FPAD + IN_N] = x
    xp = xp.reshape(NCORES, S * IN_NP)

    off = _offsets()
    idx = np.zeros((P, NB * S), dtype=np.int32)
    for k in range(NB):
        for s in range(S):
            idx[:, k * S + s] = (
                FPAD + off[k * P:(k + 1) * P] + s * IN_NP
            ).astype(np.int32)
    msk = _mask_array()
    return [{"inp": xp[c][:, None], "idx": idx, "msk": msk}
            for c in range(NCORES)]


def assemble_out(results) -> np.ndarray:
    outs = []
    for c in range(NCORES):
        o = results[c]["out"].reshape(S, OUT_NP)[:, :OUT_N]
        outs.append(o.reshape(S, MAT, MAT))
    return np.concatenate(outs, axis=0)


def kernel(inputs: np.ndarray) -> np.ndarray:
    from concourse.bass_utils import run_bass_kernel_spmd

    nc = _get_nc()
    in_maps = make_in_maps(np.asarray(inputs))
    res = run_bass_kernel_spmd(nc, in_maps, core_ids=list(range(NCORES)))
    return assemble_out(res.results)


if __name__ == "__main__":
    rng = np.random.default_rng(0)
    x = rng.standard_normal((BATCH, IN_N), dtype=np.float32)
    y = kernel(x)
    # numpy reference
    r, c = np.triu_indices(MAT)
    exp = np.zeros((BATCH, MAT, MAT), dtype=np.float32)
    exp[:, r, c] = x
    err = np.abs(y - exp).max()
    denom = max(np.abs(exp).max(), 1e-9)
    print("max abs err:", err, "rel:", err / denom)
    assert err == 0.0, "mismatch"
    print("OK")

